# revision 1
# baseline (speedup 1.0000x reference)
"""EventEmbeddingModel Trainium2 kernel.

kernel(**inputs) takes the FULL (unsharded) inputs and returns the full
[B, D] float32 output.  Data-parallel over batch across the 8 NeuronCores;
the embedding table and LinearQ weights are replicated.

v4 (ragged, bf16, dma_gather, default): host does layout only — batch
rows sorted by history length, striped across cores; each 128-row chunk's
valid slots (plus the ent_id fallback row when hist_len == 0) are packed
into an exact-fit [128, lc] skeleton, sorted by vocab id, and cut into
segments of <= GBLK layers whose vocab span fits the gather op's int16
index range (the segment's base row becomes a compile-time offset into
the bf16 emb table).  Each segment is ONE gpsimd.dma_gather instruction
of <= 1024 indices (larger faults the hw ucode; wide multi-column
indirect_dma_start is unsupported entirely).  Decay weights
w = exp(t - ct) are precomputed on host; the one-hot rhs tiles
(column-match * weight) are built ONCE in the prologue from seg/wg and
held in SBUF, so the steady-state loop is just gathers + two bf16
matmuls per group accumulating his^T in PSUM and the final linear
y = his @ W^T + b (bf16, fp32 PSUM); y is emitted bf16 and widened on
host.  Host inverse-permutes output rows.

v1 (dense fp32 fallback): every row processes all 64 slots; used only if
the v4 packing plan cannot be built.

Both validated on hardware against the jax reference.
"""
import sys

import numpy as np

if "/opt/trn_rl_repo" not in sys.path:
    sys.path.insert(0, "/opt/trn_rl_repo")

B, L, V, D = 8192, 64, 100000, 256
N_CORES = 8
BL = B // N_CORES
P = 128
NCHUNK = BL // P
NPAIR = L

GBLK = 8      # max gather layers (128 rows each) per dma_gather
              # (1024 idxs/instr: larger faults the SWDGE ring on hw)
RR = 32768    # int16-addressable vocab rows per gather segment
GP_BUFS = 6   # gather tile buffering
STP_BUFS = 2  # (stage pool now unused in the rep loop)


def _bf16(x):
    from concourse import mybir
    return np.ascontiguousarray(np.asarray(x).astype(mybir.dt.np(mybir.dt.bfloat16)))


def _common_io(nc, mybir, emb_dtype, y_dtype=None):
    f32 = mybir.dt.float32
    emb = nc.dram_tensor("emb", [V, D], emb_dtype, kind="ExternalInput").ap()
    wt_d = nc.dram_tensor("WT", [D, D], emb_dtype, kind="ExternalInput").ap()
    b_d = nc.dram_tensor("bvec", [D], f32, kind="ExternalInput").ap()
    y_d = nc.dram_tensor("y", [BL, D], y_dtype or emb_dtype,
                         kind="ExternalOutput").ap()
    return emb, wt_d, b_d, y_d


def _bias_bcast(nc, cpool, pt, b_d, mybir):
    """PE-broadcast the bias vector to a [P, D] fp32 tile."""
    f32 = mybir.dt.float32
    bias_row = cpool.tile([1, D], f32)
    nc.sync.dma_start(out=bias_row[:], in_=b_d[None, :])
    ones_row = cpool.tile([1, P], f32)
    nc.vector.memset(ones_row[:], 1.0)
    bias_ps = pt.tile([P, D], f32, tag="bias_ps")
    nc.tensor.matmul(out=bias_ps[:], lhsT=ones_row[:], rhs=bias_row[:],
                     start=True, stop=True)
    bias_t = cpool.tile([P, D], f32)
    nc.vector.tensor_copy(bias_t[:], bias_ps[:])
    return bias_t


def build_nc_v4(plan, debug=False, reps=1):
    """plan: per chunk, a tuple of (emb_base_row, n_layers) segments, or
    ("cols", n_layers) for the per-column indirect fallback."""
    import concourse.bass as bass
    import concourse.tile as tile
    from concourse import bacc, mybir

    f32, i32, i16 = mybir.dt.float32, mybir.dt.int32, mybir.dt.int16
    bf16 = mybir.dt.bfloat16
    op = mybir.AluOpType
    act = mybir.ActivationFunctionType

    nc = bacc.Bacc("TRN2", target_bir_lowering=False, debug=debug,
                   num_devices=N_CORES, dynamic_dma_scratch_size=65536)

    lc = tuple(segs[1] if segs[0] == "cols" else sum(n for _b, n in segs)
               for segs in plan)
    g_total = sum(lc)
    idx_d = nc.dram_tensor("idxg", [P, 8 * g_total], i16,
                           kind="ExternalInput").ap()
    idxc_d = nc.dram_tensor("idxc", [P, g_total], i32,
                            kind="ExternalInput").ap()
    wg_d = nc.dram_tensor("wgg", [P, g_total], f32,
                          kind="ExternalInput").ap()
    seg_d = nc.dram_tensor("segg", [P, g_total], f32,
                           kind="ExternalInput").ap()
    emb, wt_d, b_d, y_d = _common_io(nc, mybir, bf16)

    with tile.TileContext(nc) as tc:
        with tc.tile_pool(name="const", bufs=1) as cpool, \
             tc.tile_pool(name="stage", bufs=STP_BUFS) as stp, \
             tc.tile_pool(name="gather", bufs=GP_BUFS) as gp, \
             tc.tile_pool(name="outp", bufs=2) as outp, \
             tc.tile_pool(name="pt", bufs=1, space="PSUM") as pt, \
             tc.tile_pool(name="phis", bufs=2, space="PSUM") as phis, \
             tc.tile_pool(name="py", bufs=2, space="PSUM") as py:

            iota_i = cpool.tile([P, P], i32)
            nc.gpsimd.iota(iota_i[:], pattern=[[1, P]], base=0,
                           channel_multiplier=0)
            iota_b = cpool.tile([P, P], bf16)
            nc.vector.tensor_copy(iota_b[:], iota_i[:])

            wt0 = cpool.tile([P, D], bf16)
            wt1 = cpool.tile([P, D], bf16)
            nc.sync.dma_start(out=wt0[:], in_=wt_d[0:P, :])
            nc.sync.dma_start(out=wt1[:], in_=wt_d[P:D, :])
            bias_t = _bias_bcast(nc, cpool, pt, b_d, mybir)

            idxg = cpool.tile([P, 8 * g_total], i16)
            nc.sync.dma_start(out=idxg[:], in_=idx_d[:, :])
            idxc = cpool.tile([P, g_total], i32)
            nc.sync.dma_start(out=idxc[:], in_=idxc_d[:, :])
            seg = cpool.tile([P, g_total], f32)
            nc.sync.dma_start(out=seg[:], in_=seg_d[:, :])
            wg = cpool.tile([P, g_total], f32)
            nc.sync.dma_start(out=wg[:], in_=wg_d[:, :])

            # rhs tiles depend only on seg/wg: build once, reuse every rep
            rhs_all = cpool.tile([P, g_total * P], bf16)
            for j in range(g_total):
                nc.vector.tensor_scalar(
                    rhs_all[:, j * P:(j + 1) * P], iota_b[:],
                    seg[:, j:j + 1], wg[:, j:j + 1],
                    op.is_equal, op.mult)

            for _rep in range(reps):
                gbase = 0
                for c in range(NCHUNK):
                    r0, r1 = c * P, (c + 1) * P
                    lcc = lc[c]

                    # one dma_gather per (segment, <=GBLK block); col_map
                    # records (tile, tile_col0) for each chunk column.
                    # "cols" chunks (vocab span too wide for int16 windows)
                    # use one classic per-column indirect gather instead.
                    col_map = []
                    col = 0
                    if plan[c][0] == "cols":
                        for j in range(lcc):
                            g = gp.tile([P, D], bf16, tag="g1")
                            nc.gpsimd.indirect_dma_start(
                                out=g[:], out_offset=None, in_=emb[:],
                                in_offset=bass.IndirectOffsetOnAxis(
                                    ap=idxc[:, gbase + j:gbase + j + 1],
                                    axis=0))
                            col_map.append((g, 0))
                    else:
                        for base, nlay in plan[c]:
                            rows = min(RR, V - base)
                            for b0 in range(0, nlay, GBLK):
                                blk = min(GBLK, nlay - b0)
                                g = gp.tile([P, GBLK * D], bf16, tag="g")
                                icol0 = 8 * (gbase + col)
                                nc.gpsimd.dma_gather(
                                    out_ap=g[:, 0:blk * D].rearrange(
                                        "p (k d) -> p k d", d=D),
                                    in_ap=emb[base:base + rows, :],
                                    idxs_ap=idxg[:, icol0:icol0 + 8 * blk],
                                    num_idxs=blk * P,
                                    num_idxs_reg=blk * P,
                                    elem_size=D)
                                for t in range(blk):
                                    col_map.append((g, t))
                                col += blk

                    hisT0 = phis.tile([P, P], f32)
                    hisT1 = phis.tile([P, P], f32)
                    for j in range(lcc):
                        colg = gbase + j
                        g, t = col_map[j]
                        goff = t * D
                        rhs_g = rhs_all[:, colg * P:(colg + 1) * P]
                        nc.tensor.matmul(
                            out=hisT0[:], lhsT=g[:, goff:goff + P],
                            rhs=rhs_g,
                            start=(j == 0), stop=(j == lcc - 1))
                        nc.tensor.matmul(
                            out=hisT1[:], lhsT=g[:, goff + P:goff + D],
                            rhs=rhs_g,
                            start=(j == 0), stop=(j == lcc - 1))
                    gbase += lcc

                    hisT0_sb = outp.tile([P, P], bf16)
                    nc.scalar.activation(out=hisT0_sb[:], in_=hisT0[:],
                                         func=act.Copy, bias=0.0, scale=1.0)
                    hisT1_sb = outp.tile([P, P], bf16)
                    nc.scalar.activation(out=hisT1_sb[:], in_=hisT1[:],
                                         func=act.Copy, bias=0.0, scale=1.0)

                    y_ps = py.tile([P, D], f32)
                    nc.tensor.matmul(out=y_ps[:], lhsT=hisT0_sb[:],
                                     rhs=wt0[:], start=True, stop=False)
                    nc.tensor.matmul(out=y_ps[:], lhsT=hisT1_sb[:],
                                     rhs=wt1[:], start=False, stop=True)

                    y_sb = outp.tile([P, D], bf16)
                    nc.vector.tensor_tensor(out=y_sb[:], in0=y_ps[:],
                                            in1=bias_t[:], op=op.add)
                    nc.sync.dma_start(out=y_d[r0:r1, :], in_=y_sb[:])

    nc.compile()
    return nc


def pack_v4(ent_ids, current_time, hist_ids, hist_times, hist_len):
    """Host-side layout for v4.  Sort rows by history length, stripe
    across cores, pack each chunk's slots into an exact-fit [128, lc]
    skeleton sorted by vocab id, and cut into segments whose vocab span
    fits the dma_gather int16 range (RR rows).  Weights are precomputed:
    w = exp(t - ct) (1.0 for the ent_id fallback; 0 on padding).

    Returns (packs, perm, plan) or (None, None, None)."""
    hl = np.asarray(hist_len, dtype=np.int64)
    hl_adj = np.maximum(hl, 1)
    order = np.argsort(hl_adj, kind="stable")

    # exact-fit layer budget per chunk: max over cores, ceil to layers
    lc = []
    for ch in range(NCHUNK):
        need = max(int(hl_adj[order[c::N_CORES][ch * P:(ch + 1) * P]].sum())
                   for c in range(N_CORES))
        lc.append(-(-need // P))
    lc = tuple(lc)
    g_total = sum(lc)
    if max(lc) > L or g_total > 1024:
        return None, None, None

    # per-core slot streams (sorted by vocab id, padded to lc*128)
    streams = []
    for c in range(N_CORES):
        rows = order[c::N_CORES]
        core = []
        for ch in range(NCHUNK):
            lcc = lc[ch]
            cap = P * lcc
            bidx = rows[ch * P:(ch + 1) * P]
            counts = hl_adj[bidx]
            total = int(counts.sum())
            if total > cap:
                return None, None, None
            seg_s = np.repeat(np.arange(P, dtype=np.float32), counts)
            idx_parts, w_parts = [], []
            for b in bidx:
                n = hl[b]
                if n > 0:
                    idx_parts.append(hist_ids[b, :n])
                    w_parts.append(
                        np.exp(hist_times[b, :n].astype(np.float64)
                               - float(current_time[b])))
                else:
                    idx_parts.append(np.array([ent_ids[b]], np.int32))
                    w_parts.append(np.ones(1, np.float64))
            idx_s = np.concatenate(idx_parts).astype(np.int32)
            w_s = np.concatenate(w_parts).astype(np.float32)
            pad = cap - total
            if pad:
                idx_s = np.pad(idx_s, (0, pad))
                w_s = np.pad(w_s, (0, pad))
                seg_s = np.pad(seg_s, (0, pad), constant_values=-1.0)
            srt = np.argsort(idx_s, kind="stable")
            core.append((idx_s[srt], w_s[srt], seg_s[srt]))
        streams.append(core)

    # segment plan per chunk: cut the sorted stream at 128-slot boundaries
    # so that within a segment every core's ids fit a RR-row window.  The
    # segment base must cover ALL cores, so base = min over cores of the
    # first id and the window check uses the max over cores of the last.
    plan = []
    for ch in range(NCHUNK):
        lcc = lc[ch]
        segs = []
        lay = 0
        while lay < lcc:
            base = min(int(streams[c][ch][0][lay * P]) for c in range(N_CORES))
            # largest nlay such that all ids in layers [lay, lay+nlay) are
            # < base + RR for every core
            nlay = 0
            for nl in range(1, lcc - lay + 1):
                hi = max(int(streams[c][ch][0][(lay + nl) * P - 1])
                         for c in range(N_CORES))
                if hi < base + RR:
                    nlay = nl
                else:
                    break
            if nlay == 0:
                segs = None  # vocab span too wide: per-column fallback
                break
            # cut at a GBLK multiple where possible so the per-instruction
            # blocks pack full (each dma_gather is capped at GBLK layers;
            # partial tail blocks waste an instruction each)
            if nlay >= GBLK and lay + nlay < lcc:
                nlay = GBLK * (nlay // GBLK)
            segs.append((base, nlay))
            lay += nlay
        plan.append(("cols", lcc) if segs is None else tuple(segs))
    plan = tuple(plan)

    packs = []
    for c in range(N_CORES):
        idxg16 = np.zeros((P, 8 * g_total), np.int16)
        idxc32 = np.zeros((P, g_total), np.int32)
        wgg = np.zeros((P, g_total), np.float32)
        segg = np.full((P, g_total), -1.0, np.float32)
        gbase = 0
        for ch in range(NCHUNK):
            lcc = lc[ch]
            idx_s, w_s, seg_s = streams[c][ch]
            # slot m of the chunk -> (partition m%128, layer m//128)
            sl = slice(gbase, gbase + lcc)
            wgg[:, sl] = w_s.reshape(lcc, P).T
            segg[:, sl] = seg_s.reshape(lcc, P).T
            if plan[ch][0] == "cols":
                idxc32[:, sl] = idx_s.reshape(lcc, P).T
                gbase += lcc
                continue
            # int16 idx, rebased per segment, wrapped [m%16, m//16] and
            # replicated across the eight 16-partition groups
            lay = 0
            for base, nlay in plan[ch]:
                loc = idx_s[lay * P:(lay + nlay) * P].astype(np.int64) - base
                if loc.min() < 0 or loc.max() >= RR:
                    return None, None, None
                wrapped = loc.astype(np.int16).reshape(8 * nlay, 16).T
                c0 = 8 * (gbase + lay)
                idxg16[:, c0:c0 + 8 * nlay] = np.tile(wrapped, (8, 1))
                lay += nlay
            gbase += lcc
        packs.append({"idxg": idxg16, "idxc": idxc32, "wgg": wgg,
                      "segg": segg})
    return packs, order, plan


# ---------------------------------------------------------------------------
# v1 dense fallback
# ---------------------------------------------------------------------------

def build_nc_v1(debug=False, reps=1):
    import concourse.bass as bass
    import concourse.tile as tile
    from concourse import bacc, mybir
    from concourse.masks import make_identity

    f32, i32 = mybir.dt.float32, mybir.dt.int32
    op = mybir.AluOpType
    act = mybir.ActivationFunctionType

    nc = bacc.Bacc("TRN2", target_bir_lowering=False, debug=debug,
                   num_devices=N_CORES)

    ent = nc.dram_tensor("ent_ids", [BL], i32, kind="ExternalInput").ap()
    ct_d = nc.dram_tensor("current_time", [BL], f32,
                          kind="ExternalInput").ap()
    hid = nc.dram_tensor("hist_ids", [BL, L], i32, kind="ExternalInput").ap()
    ht_d = nc.dram_tensor("hist_times", [BL, L], f32,
                          kind="ExternalInput").ap()
    hl_d = nc.dram_tensor("hist_len", [BL], i32, kind="ExternalInput").ap()
    f32_ = mybir.dt.float32
    emb = nc.dram_tensor("emb", [V, D], f32_, kind="ExternalInput").ap()
    wt_d = nc.dram_tensor("WT", [D, D], f32_, kind="ExternalInput").ap()
    b_d = nc.dram_tensor("bvec", [D], f32_, kind="ExternalInput").ap()
    y_d = nc.dram_tensor("y", [BL, D], f32_, kind="ExternalOutput").ap()

    with tile.TileContext(nc) as tc:
        with tc.tile_pool(name="const", bufs=1) as cpool, \
             tc.tile_pool(name="io", bufs=2) as iop, \
             tc.tile_pool(name="stage", bufs=2) as stp, \
             tc.tile_pool(name="gather", bufs=8) as gp, \
             tc.tile_pool(name="outp", bufs=2) as outp, \
             tc.tile_pool(name="pt", bufs=1, space="PSUM") as pt, \
             tc.tile_pool(name="phis", bufs=2, space="PSUM") as phis, \
             tc.tile_pool(name="py", bufs=2, space="PSUM") as py:

            ident = cpool.tile([P, P], f32)
            make_identity(nc, ident[:])

            iota64_i = cpool.tile([P, L], i32)
            nc.gpsimd.iota(iota64_i[:], pattern=[[1, L]], base=0,
                           channel_multiplier=0)
            iota64_f = cpool.tile([P, L], f32)
            nc.vector.tensor_copy(iota64_f[:], iota64_i[:])

            iotap_i = cpool.tile([P, 1], i32)
            nc.gpsimd.iota(iotap_i[:], pattern=[[0, 1]], base=0,
                           channel_multiplier=1)
            iotap_f = cpool.tile([P, 1], f32)
            nc.vector.tensor_copy(iotap_f[:], iotap_i[:])

            halfmask = cpool.tile([P, 2], f32)
            nc.vector.tensor_scalar(halfmask[:, 0:1], iotap_f[:], 64.0, None,
                                    op.is_lt)
            nc.vector.tensor_scalar(halfmask[:, 1:2], iotap_f[:], 63.0, None,
                                    op.is_gt)

            wt0 = cpool.tile([P, D], f32)
            wt1 = cpool.tile([P, D], f32)
            nc.sync.dma_start(out=wt0[:], in_=wt_d[0:P, :])
            nc.sync.dma_start(out=wt1[:], in_=wt_d[P:D, :])
            bias_t = _bias_bcast(nc, cpool, pt, b_d, mybir)

            for _rep in range(reps):
                for c in range(NCHUNK):
                    r0, r1 = c * P, (c + 1) * P

                    idx_nat = iop.tile([P, L], i32)
                    nc.sync.dma_start(out=idx_nat[:], in_=hid[r0:r1, :])
                    ht = iop.tile([P, L], f32)
                    nc.sync.dma_start(out=ht[:], in_=ht_d[r0:r1, :])
                    ct = iop.tile([P, 1], f32)
                    nc.sync.dma_start(out=ct[:], in_=ct_d[r0:r1, None])
                    hl_i = iop.tile([P, 1], i32)
                    nc.sync.dma_start(out=hl_i[:], in_=hl_d[r0:r1, None])
                    eid = iop.tile([P, 1], i32)
                    nc.sync.dma_start(out=eid[:], in_=ent[r0:r1, None])

                    nct = stp.tile([P, 1], f32)
                    nc.vector.tensor_scalar_mul(nct[:], ct[:], -1.0)
                    hl_f = stp.tile([P, 1], f32)
                    nc.vector.tensor_copy(hl_f[:], hl_i[:])

                    wdup = stp.tile([P, 2 * L], f32)
                    nc.scalar.activation(out=wdup[:, 0:L], in_=ht[:],
                                         func=act.Exp, bias=nct[:], scale=1.0)
                    mask = stp.tile([P, L], f32)
                    nc.vector.tensor_scalar(mask[:], iota64_f[:], hl_f[:],
                                            None, op.is_lt)
                    nc.vector.tensor_tensor(out=wdup[:, 0:L],
                                            in0=wdup[:, 0:L], in1=mask[:],
                                            op=op.mult)
                    m_f = stp.tile([P, 1], f32)
                    nc.vector.tensor_scalar(m_f[:], hl_f[:], 0.0, None,
                                            op.is_equal)
                    nc.vector.tensor_tensor(out=wdup[:, 0:1],
                                            in0=wdup[:, 0:1], in1=m_f[:],
                                            op=op.add)
                    nc.vector.tensor_copy(wdup[:, L:2 * L], wdup[:, 0:L])

                    m_i = stp.tile([P, 1], i32)
                    nc.vector.tensor_scalar(m_i[:], hl_i[:], 0, None,
                                            op.is_equal)
                    nc.vector.copy_predicated(out=idx_nat[:, 0:1],
                                              mask=m_i[:], data=eid[:])

                    idxdup = stp.tile([P, 2 * L], f32)
                    nc.vector.tensor_copy(idxdup[:, 0:L], idx_nat[:])
                    nc.vector.tensor_copy(idxdup[:, L:2 * L], idx_nat[:])

                    t_w = pt.tile([P, P], f32, tag="tw")
                    nc.tensor.transpose(out=t_w[:], in_=wdup[:],
                                        identity=ident[:])
                    t_i = pt.tile([P, P], f32, tag="ti")
                    nc.tensor.transpose(out=t_i[:], in_=idxdup[:],
                                        identity=ident[:])

                    w_shuf = stp.tile([P, L], f32)
                    nc.vector.tensor_copy(w_shuf[0:64, :], t_w[0:64, 0:P:2])
                    nc.vector.tensor_copy(w_shuf[64:P, :], t_w[64:P, 1:P:2])
                    idx_shuf_f = stp.tile([P, L], f32)
                    nc.vector.tensor_copy(idx_shuf_f[0:64, :],
                                          t_i[0:64, 0:P:2])
                    nc.vector.tensor_copy(idx_shuf_f[64:P, :],
                                          t_i[64:P, 1:P:2])
                    idx_shuf = stp.tile([P, L], i32)
                    nc.vector.tensor_copy(idx_shuf[:], idx_shuf_f[:])

                    rhs_full = stp.tile([P, 2 * L], f32)
                    nc.vector.tensor_tensor(
                        out=rhs_full[:].rearrange("p (j n) -> p j n", n=2),
                        in0=w_shuf[:, :, None].to_broadcast([P, L, 2]),
                        in1=halfmask[:, None, :].to_broadcast([P, L, 2]),
                        op=op.mult)

                    hisT0 = phis.tile([P, P], f32)
                    hisT1 = phis.tile([P, P], f32)

                    for J in range(NPAIR):
                        g = gp.tile([P, D], f32, tag="g")
                        nc.gpsimd.indirect_dma_start(
                            out=g[:], out_offset=None, in_=emb[:],
                            in_offset=bass.IndirectOffsetOnAxis(
                                ap=idx_shuf[:, J:J + 1], axis=0))
                        nc.tensor.matmul(
                            out=hisT0[:, 2 * J:2 * J + 2], lhsT=g[:, 0:P],
                            rhs=rhs_full[:, 2 * J:2 * J + 2],
                            start=True, stop=True)
                        nc.tensor.matmul(
                            out=hisT1[:, 2 * J:2 * J + 2], lhsT=g[:, P:D],
                            rhs=rhs_full[:, 2 * J:2 * J + 2],
                            start=True, stop=True)

                    hisT0_sb = outp.tile([P, P], f32)
                    nc.vector.tensor_copy(hisT0_sb[:], hisT0[:])
                    hisT1_sb = outp.tile([P, P], f32)
                    nc.vector.tensor_copy(hisT1_sb[:], hisT1[:])

                    y_ps = py.tile([P, D], f32)
                    nc.tensor.matmul(out=y_ps[:], lhsT=hisT0_sb[:],
                                     rhs=wt0[:], start=True, stop=False)
                    nc.tensor.matmul(out=y_ps[:], lhsT=hisT1_sb[:],
                                     rhs=wt1[:], start=False, stop=True)

                    y_sb = outp.tile([P, D], f32)
                    nc.vector.tensor_tensor(out=y_sb[:], in0=y_ps[:],
                                            in1=bias_t[:], op=op.add)
                    nc.sync.dma_start(out=y_d[r0:r1, :], in_=y_sb[:])

    nc.compile()
    return nc


_NC_CACHE = {}


def _get_nc(which, plan=None):
    key = (which, plan)
    if key not in _NC_CACHE:
        _NC_CACHE[key] = (build_nc_v4(plan) if which == "v4"
                          else build_nc_v1())
    return _NC_CACHE[key]


def _norm_inputs(ent_ids, current_time, hist_ids, hist_times, hist_len,
                 emb, W, b):
    return (
        np.ascontiguousarray(np.asarray(ent_ids, dtype=np.int32)),
        np.ascontiguousarray(np.asarray(current_time, np.float32)),
        np.ascontiguousarray(np.asarray(hist_ids, dtype=np.int32)),
        np.ascontiguousarray(np.asarray(hist_times, np.float32)),
        np.ascontiguousarray(np.asarray(hist_len, dtype=np.int32)),
        np.ascontiguousarray(np.asarray(emb, dtype=np.float32)),
        np.ascontiguousarray(np.asarray(W, dtype=np.float32)),
        np.ascontiguousarray(np.asarray(b, dtype=np.float32)),
    )


def make_in_maps(ent_ids, current_time, hist_ids, hist_times, hist_len,
                 emb, W, b):
    """v1 (dense) per-core input maps."""
    ent_ids, current_time, hist_ids, hist_times, hist_len, emb, W, b = \
        _norm_inputs(ent_ids, current_time, hist_ids, hist_times, hist_len,
                     emb, W, b)
    WT = np.ascontiguousarray(W.T)
    in_maps = []
    for c in range(N_CORES):
        s = slice(c * BL, (c + 1) * BL)
        in_maps.append({
            "ent_ids": ent_ids[s], "current_time": current_time[s],
            "hist_ids": hist_ids[s], "hist_times": hist_times[s],
            "hist_len": hist_len[s], "emb": emb, "WT": WT, "bvec": b,
        })
    return in_maps


def make_in_maps_v4(ent_ids, current_time, hist_ids, hist_times, hist_len,
                    emb, W, b):
    """v4 (ragged bf16) per-core input maps + output permutation + plan."""
    ent_ids, current_time, hist_ids, hist_times, hist_len, emb, W, b = \
        _norm_inputs(ent_ids, current_time, hist_ids, hist_times, hist_len,
                     emb, W, b)
    packs, order, plan = pack_v4(ent_ids, current_time, hist_ids, hist_times,
                                 hist_len)
    if packs is None:
        return None, None, None
    emb_b = _bf16(emb)
    wt_b = _bf16(W.T)
    in_maps = []
    for c in range(N_CORES):
        pk = packs[c]
        in_maps.append({
            "idxg": pk["idxg"], "idxc": pk["idxc"], "wgg": pk["wgg"],
            "segg": pk["segg"],
            "emb": emb_b, "WT": wt_b, "bvec": b,
        })
    return in_maps, order, plan


def kernel(ent_ids, current_time, hist_ids, hist_times, hist_len, emb, W, b):
    from concourse.bass_utils import run_bass_kernel_spmd

    args = (ent_ids, current_time, hist_ids, hist_times, hist_len, emb, W, b)
    in_maps, order, plan = make_in_maps_v4(*args)
    if in_maps is not None:
        nc = _get_nc("v4", plan)
        res = run_bass_kernel_spmd(nc, in_maps, list(range(N_CORES)))
        y_sorted = np.stack([
            np.asarray(res.results[c]["y"], np.float32)
            for c in range(N_CORES)])
        # core c position p holds batch row order[8p + c]
        y_full = np.empty((B, D), np.float32)
        pos = np.arange(BL)
        for c in range(N_CORES):
            y_full[order[N_CORES * pos + c]] = y_sorted[c]
        return y_full

    nc = _get_nc("v1")
    res = run_bass_kernel_spmd(nc, make_in_maps(*args),
                               list(range(N_CORES)))
    return np.concatenate([res.results[c]["y"] for c in range(N_CORES)],
                          axis=0)



# revision 7
# speedup vs baseline: 2.0856x; 2.0856x over previous
"""EventEmbeddingModel Trainium2 kernel.

kernel(**inputs) takes the FULL (unsharded) inputs and returns the full
[B, D] float32 output.  Data-parallel over batch across the 8 NeuronCores;
the embedding table and LinearQ weights are replicated.

v4 (ragged, bf16, dma_gather, default): host does layout only — batch
rows sorted by history length, striped across cores; each 128-row chunk's
valid slots (plus the ent_id fallback row when hist_len == 0) are packed
into an exact-fit [128, lc] skeleton, sorted by vocab id, and cut into
segments of <= GBLK layers whose vocab span fits the gather op's int16
index range (the segment's base row becomes a compile-time offset into
the bf16 emb table).  Each segment is ONE gpsimd.dma_gather instruction
of <= 1024 indices (larger faults the hw ucode; wide multi-column
indirect_dma_start is unsupported entirely).  Decay weights
w = exp(t - ct) are precomputed on host; the one-hot rhs tiles
(column-match * weight) are built ONCE in the prologue from seg/wg and
held in SBUF, so the steady-state loop is just gathers + two bf16
matmuls per group accumulating his^T in PSUM and the final linear
y = his @ W^T + b (bf16, fp32 PSUM); y is emitted bf16 and widened on
host.  Host inverse-permutes output rows.

v1 (dense fp32 fallback): every row processes all 64 slots; used only if
the v4 packing plan cannot be built.

Both validated on hardware against the jax reference.
"""
import sys

import numpy as np

if "/opt/trn_rl_repo" not in sys.path:
    sys.path.insert(0, "/opt/trn_rl_repo")

B, L, V, D = 8192, 64, 100000, 256
N_CORES = 8
BL = B // N_CORES
P = 128
NCHUNK = BL // P
NPAIR = L

GBLK = 8      # max gather layers (128 rows each) per dma_gather
              # (1024 idxs/instr: larger faults the SWDGE ring on hw)
RR = 32768    # int16-addressable vocab rows per gather segment
GP_BUFS = 4   # gather tile buffering
NQ = 4        # SWDGE queues: gather descriptor processing parallelizes
              # almost linearly across queues (276us -> 82us measured)
SCRATCH = 131072  # dynamic DMA scratch bytes/partition (ring space for NQ)


def _bf16(x):
    from concourse import mybir
    return np.ascontiguousarray(np.asarray(x).astype(mybir.dt.np(mybir.dt.bfloat16)))


def _common_io(nc, mybir, emb_dtype, y_dtype=None):
    f32 = mybir.dt.float32
    emb = nc.dram_tensor("emb", [V, D], emb_dtype, kind="ExternalInput").ap()
    wt_d = nc.dram_tensor("WT", [D, D], emb_dtype, kind="ExternalInput").ap()
    b_d = nc.dram_tensor("bvec", [D], f32, kind="ExternalInput").ap()
    y_d = nc.dram_tensor("y", [BL, D], y_dtype or emb_dtype,
                         kind="ExternalOutput").ap()
    return emb, wt_d, b_d, y_d


def _bias_bcast(nc, cpool, pt, b_d, mybir):
    """PE-broadcast the bias vector to a [P, D] fp32 tile."""
    f32 = mybir.dt.float32
    bias_row = cpool.tile([1, D], f32)
    nc.sync.dma_start(out=bias_row[:], in_=b_d[None, :])
    ones_row = cpool.tile([1, P], f32)
    nc.vector.memset(ones_row[:], 1.0)
    bias_ps = pt.tile([P, D], f32, tag="bias_ps")
    nc.tensor.matmul(out=bias_ps[:], lhsT=ones_row[:], rhs=bias_row[:],
                     start=True, stop=True)
    bias_t = cpool.tile([P, D], f32)
    nc.vector.tensor_copy(bias_t[:], bias_ps[:])
    return bias_t


def build_nc_v4(plan, debug=False, reps=1):
    """plan: per chunk, a tuple of (emb_base_row, n_layers) segments, or
    ("cols", n_layers) for the per-column indirect fallback."""
    import concourse.bass as bass
    import concourse.tile as tile
    from concourse import bacc, mybir

    f32, i32, i16 = mybir.dt.float32, mybir.dt.int32, mybir.dt.int16
    bf16 = mybir.dt.bfloat16
    op = mybir.AluOpType
    act = mybir.ActivationFunctionType

    nc = bacc.Bacc("TRN2", target_bir_lowering=False, debug=debug,
                   num_devices=N_CORES, dynamic_dma_scratch_size=SCRATCH,
                   num_swdge_queues=NQ)

    lc = tuple(segs[1] if segs[0] == "cols" else sum(n for _b, n in segs)
               for segs in plan)
    g_total = sum(lc)
    idx_d = nc.dram_tensor("idxg", [P, 8 * g_total], i16,
                           kind="ExternalInput").ap()
    idxc_d = nc.dram_tensor("idxc", [P, g_total], i32,
                            kind="ExternalInput").ap()
    wg_d = nc.dram_tensor("wgg", [P, g_total], f32,
                          kind="ExternalInput").ap()
    seg_d = nc.dram_tensor("segg", [P, g_total], f32,
                           kind="ExternalInput").ap()
    emb, wt_d, b_d, y_d = _common_io(nc, mybir, bf16)

    with tile.TileContext(nc) as tc:
        with tc.tile_pool(name="const", bufs=1) as cpool, \
             tc.tile_pool(name="gather", bufs=GP_BUFS) as gp, \
             tc.tile_pool(name="outp", bufs=1) as outp, \
             tc.tile_pool(name="pt", bufs=1, space="PSUM") as pt, \
             tc.tile_pool(name="phis", bufs=2, space="PSUM") as phis, \
             tc.tile_pool(name="py", bufs=2, space="PSUM") as py:

            wt0 = cpool.tile([P, D], bf16)
            wt1 = cpool.tile([P, D], bf16)
            nc.sync.dma_start(out=wt0[:], in_=wt_d[0:P, :])
            nc.sync.dma_start(out=wt1[:], in_=wt_d[P:D, :])
            bias_t = _bias_bcast(nc, cpool, pt, b_d, mybir)

            idxg = cpool.tile([P, 8 * g_total], i16)
            nc.sync.dma_start(out=idxg[:], in_=idx_d[:, :])
            idxc = cpool.tile([P, g_total], i32)
            nc.sync.dma_start(out=idxc[:], in_=idxc_d[:, :])

            # rhs tiles depend only on seg/wg: build once, reuse every rep.
            # The seg/wg/iota temporaries live in a scoped pool so their
            # SBUF frees before the steady-state loop.
            rhs_all = cpool.tile([P, g_total * P], bf16)
            with tc.tile_pool(name="prolog", bufs=1) as ppool:
                iota_i = ppool.tile([P, P], i32)
                nc.gpsimd.iota(iota_i[:], pattern=[[1, P]], base=0,
                               channel_multiplier=0)
                iota_b = ppool.tile([P, P], bf16)
                nc.vector.tensor_copy(iota_b[:], iota_i[:])
                seg = ppool.tile([P, g_total], f32)
                nc.sync.dma_start(out=seg[:], in_=seg_d[:, :])
                wg = ppool.tile([P, g_total], f32)
                nc.sync.dma_start(out=wg[:], in_=wg_d[:, :])
                for j in range(g_total):
                    nc.vector.tensor_scalar(
                        rhs_all[:, j * P:(j + 1) * P], iota_b[:],
                        seg[:, j:j + 1], wg[:, j:j + 1],
                        op.is_equal, op.mult)

            qi = 0  # SWDGE queue round-robin counter
            for _rep in range(reps):
                gbase = 0
                for c in range(NCHUNK):
                    r0, r1 = c * P, (c + 1) * P
                    lcc = lc[c]

                    # one dma_gather per (segment, <=GBLK block); col_map
                    # records (tile, tile_col0) for each chunk column.
                    # "cols" chunks (vocab span too wide for int16 windows)
                    # use one classic per-column indirect gather instead.
                    col_map = []
                    col = 0
                    if plan[c][0] == "cols":
                        for j in range(lcc):
                            g = gp.tile([P, D], bf16, tag="g1")
                            nc.gpsimd.indirect_dma_start(
                                out=g[:], out_offset=None, in_=emb[:],
                                in_offset=bass.IndirectOffsetOnAxis(
                                    ap=idxc[:, gbase + j:gbase + j + 1],
                                    axis=0))
                            col_map.append((g, 0))
                    else:
                        for base, nlay in plan[c]:
                            rows = min(RR, V - base)
                            for b0 in range(0, nlay, GBLK):
                                blk = min(GBLK, nlay - b0)
                                g = gp.tile([P, GBLK * D], bf16, tag="g")
                                icol0 = 8 * (gbase + col)
                                nc.gpsimd.dma_gather(
                                    out_ap=g[:, 0:blk * D].rearrange(
                                        "p (k d) -> p k d", d=D),
                                    in_ap=emb[base:base + rows, :],
                                    idxs_ap=idxg[:, icol0:icol0 + 8 * blk],
                                    num_idxs=blk * P,
                                    num_idxs_reg=blk * P,
                                    elem_size=D,
                                    queue_num=qi % NQ)
                                qi += 1
                                for t in range(blk):
                                    col_map.append((g, t))
                                col += blk

                    hisT0 = phis.tile([P, P], f32)
                    hisT1 = phis.tile([P, P], f32)
                    for j in range(lcc):
                        colg = gbase + j
                        g, t = col_map[j]
                        goff = t * D
                        rhs_g = rhs_all[:, colg * P:(colg + 1) * P]
                        nc.tensor.matmul(
                            out=hisT0[:], lhsT=g[:, goff:goff + P],
                            rhs=rhs_g,
                            start=(j == 0), stop=(j == lcc - 1))
                        nc.tensor.matmul(
                            out=hisT1[:], lhsT=g[:, goff + P:goff + D],
                            rhs=rhs_g,
                            start=(j == 0), stop=(j == lcc - 1))
                    gbase += lcc

                    hisT0_sb = outp.tile([P, P], bf16)
                    nc.scalar.activation(out=hisT0_sb[:], in_=hisT0[:],
                                         func=act.Copy, bias=0.0, scale=1.0)
                    hisT1_sb = outp.tile([P, P], bf16)
                    nc.scalar.activation(out=hisT1_sb[:], in_=hisT1[:],
                                         func=act.Copy, bias=0.0, scale=1.0)

                    y_ps = py.tile([P, D], f32)
                    nc.tensor.matmul(out=y_ps[:], lhsT=hisT0_sb[:],
                                     rhs=wt0[:], start=True, stop=False)
                    nc.tensor.matmul(out=y_ps[:], lhsT=hisT1_sb[:],
                                     rhs=wt1[:], start=False, stop=True)

                    y_sb = outp.tile([P, D], bf16)
                    nc.vector.tensor_tensor(out=y_sb[:], in0=y_ps[:],
                                            in1=bias_t[:], op=op.add)
                    nc.sync.dma_start(out=y_d[r0:r1, :], in_=y_sb[:])

    nc.compile()
    return nc


def pack_v4(ent_ids, current_time, hist_ids, hist_times, hist_len):
    """Host-side layout for v4.  Sort rows by history length, stripe
    across cores, pack each chunk's slots into an exact-fit [128, lc]
    skeleton sorted by vocab id, and cut into segments whose vocab span
    fits the dma_gather int16 range (RR rows).  Weights are precomputed:
    w = exp(t - ct) (1.0 for the ent_id fallback; 0 on padding).

    Returns (packs, perm, plan) or (None, None, None)."""
    hl = np.asarray(hist_len, dtype=np.int64)
    hl_adj = np.maximum(hl, 1)
    order = np.argsort(hl_adj, kind="stable")

    # exact-fit layer budget per chunk: max over cores, ceil to layers
    lc = []
    for ch in range(NCHUNK):
        need = max(int(hl_adj[order[c::N_CORES][ch * P:(ch + 1) * P]].sum())
                   for c in range(N_CORES))
        lc.append(-(-need // P))
    lc = tuple(lc)
    g_total = sum(lc)
    if max(lc) > L or g_total > 1024:
        return None, None, None

    # per-core slot streams (sorted by vocab id, padded to lc*128)
    streams = []
    for c in range(N_CORES):
        rows = order[c::N_CORES]
        core = []
        for ch in range(NCHUNK):
            lcc = lc[ch]
            cap = P * lcc
            bidx = rows[ch * P:(ch + 1) * P]
            counts = hl_adj[bidx]
            total = int(counts.sum())
            if total > cap:
                return None, None, None
            seg_s = np.repeat(np.arange(P, dtype=np.float32), counts)
            idx_parts, w_parts = [], []
            for b in bidx:
                n = hl[b]
                if n > 0:
                    idx_parts.append(hist_ids[b, :n])
                    w_parts.append(
                        np.exp(hist_times[b, :n].astype(np.float64)
                               - float(current_time[b])))
                else:
                    idx_parts.append(np.array([ent_ids[b]], np.int32))
                    w_parts.append(np.ones(1, np.float64))
            idx_s = np.concatenate(idx_parts).astype(np.int32)
            w_s = np.concatenate(w_parts).astype(np.float32)
            pad = cap - total
            if pad:
                idx_s = np.pad(idx_s, (0, pad))
                w_s = np.pad(w_s, (0, pad))
                seg_s = np.pad(seg_s, (0, pad), constant_values=-1.0)
            srt = np.argsort(idx_s, kind="stable")
            core.append((idx_s[srt], w_s[srt], seg_s[srt]))
        streams.append(core)

    # segment plan per chunk: cut the sorted stream at 128-slot boundaries
    # so that within a segment every core's ids fit a RR-row window.  The
    # segment base must cover ALL cores, so base = min over cores of the
    # first id and the window check uses the max over cores of the last.
    plan = []
    for ch in range(NCHUNK):
        lcc = lc[ch]
        segs = []
        lay = 0
        while lay < lcc:
            base = min(int(streams[c][ch][0][lay * P]) for c in range(N_CORES))
            # largest nlay such that all ids in layers [lay, lay+nlay) are
            # < base + RR for every core
            nlay = 0
            for nl in range(1, lcc - lay + 1):
                hi = max(int(streams[c][ch][0][(lay + nl) * P - 1])
                         for c in range(N_CORES))
                if hi < base + RR:
                    nlay = nl
                else:
                    break
            if nlay == 0:
                segs = None  # vocab span too wide: per-column fallback
                break
            # cut at a GBLK multiple where possible so the per-instruction
            # blocks pack full (each dma_gather is capped at GBLK layers;
            # partial tail blocks waste an instruction each)
            if nlay >= GBLK and lay + nlay < lcc:
                nlay = GBLK * (nlay // GBLK)
            segs.append((base, nlay))
            lay += nlay
        plan.append(("cols", lcc) if segs is None else tuple(segs))
    plan = tuple(plan)

    packs = []
    for c in range(N_CORES):
        idxg16 = np.zeros((P, 8 * g_total), np.int16)
        idxc32 = np.zeros((P, g_total), np.int32)
        wgg = np.zeros((P, g_total), np.float32)
        segg = np.full((P, g_total), -1.0, np.float32)
        gbase = 0
        for ch in range(NCHUNK):
            lcc = lc[ch]
            idx_s, w_s, seg_s = streams[c][ch]
            # slot m of the chunk -> (partition m%128, layer m//128)
            sl = slice(gbase, gbase + lcc)
            wgg[:, sl] = w_s.reshape(lcc, P).T
            segg[:, sl] = seg_s.reshape(lcc, P).T
            if plan[ch][0] == "cols":
                idxc32[:, sl] = idx_s.reshape(lcc, P).T
                gbase += lcc
                continue
            # int16 idx, rebased per segment, wrapped [m%16, m//16] and
            # replicated across the eight 16-partition groups
            lay = 0
            for base, nlay in plan[ch]:
                loc = idx_s[lay * P:(lay + nlay) * P].astype(np.int64) - base
                if loc.min() < 0 or loc.max() >= RR:
                    return None, None, None
                wrapped = loc.astype(np.int16).reshape(8 * nlay, 16).T
                c0 = 8 * (gbase + lay)
                idxg16[:, c0:c0 + 8 * nlay] = np.tile(wrapped, (8, 1))
                lay += nlay
            gbase += lcc
        packs.append({"idxg": idxg16, "idxc": idxc32, "wgg": wgg,
                      "segg": segg})
    return packs, order, plan


# ---------------------------------------------------------------------------
# v1 dense fallback
# ---------------------------------------------------------------------------

def build_nc_v1(debug=False, reps=1):
    import concourse.bass as bass
    import concourse.tile as tile
    from concourse import bacc, mybir
    from concourse.masks import make_identity

    f32, i32 = mybir.dt.float32, mybir.dt.int32
    op = mybir.AluOpType
    act = mybir.ActivationFunctionType

    nc = bacc.Bacc("TRN2", target_bir_lowering=False, debug=debug,
                   num_devices=N_CORES)

    ent = nc.dram_tensor("ent_ids", [BL], i32, kind="ExternalInput").ap()
    ct_d = nc.dram_tensor("current_time", [BL], f32,
                          kind="ExternalInput").ap()
    hid = nc.dram_tensor("hist_ids", [BL, L], i32, kind="ExternalInput").ap()
    ht_d = nc.dram_tensor("hist_times", [BL, L], f32,
                          kind="ExternalInput").ap()
    hl_d = nc.dram_tensor("hist_len", [BL], i32, kind="ExternalInput").ap()
    f32_ = mybir.dt.float32
    emb = nc.dram_tensor("emb", [V, D], f32_, kind="ExternalInput").ap()
    wt_d = nc.dram_tensor("WT", [D, D], f32_, kind="ExternalInput").ap()
    b_d = nc.dram_tensor("bvec", [D], f32_, kind="ExternalInput").ap()
    y_d = nc.dram_tensor("y", [BL, D], f32_, kind="ExternalOutput").ap()

    with tile.TileContext(nc) as tc:
        with tc.tile_pool(name="const", bufs=1) as cpool, \
             tc.tile_pool(name="io", bufs=2) as iop, \
             tc.tile_pool(name="stage", bufs=2) as stp, \
             tc.tile_pool(name="gather", bufs=8) as gp, \
             tc.tile_pool(name="outp", bufs=2) as outp, \
             tc.tile_pool(name="pt", bufs=1, space="PSUM") as pt, \
             tc.tile_pool(name="phis", bufs=2, space="PSUM") as phis, \
             tc.tile_pool(name="py", bufs=2, space="PSUM") as py:

            ident = cpool.tile([P, P], f32)
            make_identity(nc, ident[:])

            iota64_i = cpool.tile([P, L], i32)
            nc.gpsimd.iota(iota64_i[:], pattern=[[1, L]], base=0,
                           channel_multiplier=0)
            iota64_f = cpool.tile([P, L], f32)
            nc.vector.tensor_copy(iota64_f[:], iota64_i[:])

            iotap_i = cpool.tile([P, 1], i32)
            nc.gpsimd.iota(iotap_i[:], pattern=[[0, 1]], base=0,
                           channel_multiplier=1)
            iotap_f = cpool.tile([P, 1], f32)
            nc.vector.tensor_copy(iotap_f[:], iotap_i[:])

            halfmask = cpool.tile([P, 2], f32)
            nc.vector.tensor_scalar(halfmask[:, 0:1], iotap_f[:], 64.0, None,
                                    op.is_lt)
            nc.vector.tensor_scalar(halfmask[:, 1:2], iotap_f[:], 63.0, None,
                                    op.is_gt)

            wt0 = cpool.tile([P, D], f32)
            wt1 = cpool.tile([P, D], f32)
            nc.sync.dma_start(out=wt0[:], in_=wt_d[0:P, :])
            nc.sync.dma_start(out=wt1[:], in_=wt_d[P:D, :])
            bias_t = _bias_bcast(nc, cpool, pt, b_d, mybir)

            for _rep in range(reps):
                for c in range(NCHUNK):
                    r0, r1 = c * P, (c + 1) * P

                    idx_nat = iop.tile([P, L], i32)
                    nc.sync.dma_start(out=idx_nat[:], in_=hid[r0:r1, :])
                    ht = iop.tile([P, L], f32)
                    nc.sync.dma_start(out=ht[:], in_=ht_d[r0:r1, :])
                    ct = iop.tile([P, 1], f32)
                    nc.sync.dma_start(out=ct[:], in_=ct_d[r0:r1, None])
                    hl_i = iop.tile([P, 1], i32)
                    nc.sync.dma_start(out=hl_i[:], in_=hl_d[r0:r1, None])
                    eid = iop.tile([P, 1], i32)
                    nc.sync.dma_start(out=eid[:], in_=ent[r0:r1, None])

                    nct = stp.tile([P, 1], f32)
                    nc.vector.tensor_scalar_mul(nct[:], ct[:], -1.0)
                    hl_f = stp.tile([P, 1], f32)
                    nc.vector.tensor_copy(hl_f[:], hl_i[:])

                    wdup = stp.tile([P, 2 * L], f32)
                    nc.scalar.activation(out=wdup[:, 0:L], in_=ht[:],
                                         func=act.Exp, bias=nct[:], scale=1.0)
                    mask = stp.tile([P, L], f32)
                    nc.vector.tensor_scalar(mask[:], iota64_f[:], hl_f[:],
                                            None, op.is_lt)
                    nc.vector.tensor_tensor(out=wdup[:, 0:L],
                                            in0=wdup[:, 0:L], in1=mask[:],
                                            op=op.mult)
                    m_f = stp.tile([P, 1], f32)
                    nc.vector.tensor_scalar(m_f[:], hl_f[:], 0.0, None,
                                            op.is_equal)
                    nc.vector.tensor_tensor(out=wdup[:, 0:1],
                                            in0=wdup[:, 0:1], in1=m_f[:],
                                            op=op.add)
                    nc.vector.tensor_copy(wdup[:, L:2 * L], wdup[:, 0:L])

                    m_i = stp.tile([P, 1], i32)
                    nc.vector.tensor_scalar(m_i[:], hl_i[:], 0, None,
                                            op.is_equal)
                    nc.vector.copy_predicated(out=idx_nat[:, 0:1],
                                              mask=m_i[:], data=eid[:])

                    idxdup = stp.tile([P, 2 * L], f32)
                    nc.vector.tensor_copy(idxdup[:, 0:L], idx_nat[:])
                    nc.vector.tensor_copy(idxdup[:, L:2 * L], idx_nat[:])

                    t_w = pt.tile([P, P], f32, tag="tw")
                    nc.tensor.transpose(out=t_w[:], in_=wdup[:],
                                        identity=ident[:])
                    t_i = pt.tile([P, P], f32, tag="ti")
                    nc.tensor.transpose(out=t_i[:], in_=idxdup[:],
                                        identity=ident[:])

                    w_shuf = stp.tile([P, L], f32)
                    nc.vector.tensor_copy(w_shuf[0:64, :], t_w[0:64, 0:P:2])
                    nc.vector.tensor_copy(w_shuf[64:P, :], t_w[64:P, 1:P:2])
                    idx_shuf_f = stp.tile([P, L], f32)
                    nc.vector.tensor_copy(idx_shuf_f[0:64, :],
                                          t_i[0:64, 0:P:2])
                    nc.vector.tensor_copy(idx_shuf_f[64:P, :],
                                          t_i[64:P, 1:P:2])
                    idx_shuf = stp.tile([P, L], i32)
                    nc.vector.tensor_copy(idx_shuf[:], idx_shuf_f[:])

                    rhs_full = stp.tile([P, 2 * L], f32)
                    nc.vector.tensor_tensor(
                        out=rhs_full[:].rearrange("p (j n) -> p j n", n=2),
                        in0=w_shuf[:, :, None].to_broadcast([P, L, 2]),
                        in1=halfmask[:, None, :].to_broadcast([P, L, 2]),
                        op=op.mult)

                    hisT0 = phis.tile([P, P], f32)
                    hisT1 = phis.tile([P, P], f32)

                    for J in range(NPAIR):
                        g = gp.tile([P, D], f32, tag="g")
                        nc.gpsimd.indirect_dma_start(
                            out=g[:], out_offset=None, in_=emb[:],
                            in_offset=bass.IndirectOffsetOnAxis(
                                ap=idx_shuf[:, J:J + 1], axis=0))
                        nc.tensor.matmul(
                            out=hisT0[:, 2 * J:2 * J + 2], lhsT=g[:, 0:P],
                            rhs=rhs_full[:, 2 * J:2 * J + 2],
                            start=True, stop=True)
                        nc.tensor.matmul(
                            out=hisT1[:, 2 * J:2 * J + 2], lhsT=g[:, P:D],
                            rhs=rhs_full[:, 2 * J:2 * J + 2],
                            start=True, stop=True)

                    hisT0_sb = outp.tile([P, P], f32)
                    nc.vector.tensor_copy(hisT0_sb[:], hisT0[:])
                    hisT1_sb = outp.tile([P, P], f32)
                    nc.vector.tensor_copy(hisT1_sb[:], hisT1[:])

                    y_ps = py.tile([P, D], f32)
                    nc.tensor.matmul(out=y_ps[:], lhsT=hisT0_sb[:],
                                     rhs=wt0[:], start=True, stop=False)
                    nc.tensor.matmul(out=y_ps[:], lhsT=hisT1_sb[:],
                                     rhs=wt1[:], start=False, stop=True)

                    y_sb = outp.tile([P, D], f32)
                    nc.vector.tensor_tensor(out=y_sb[:], in0=y_ps[:],
                                            in1=bias_t[:], op=op.add)
                    nc.sync.dma_start(out=y_d[r0:r1, :], in_=y_sb[:])

    nc.compile()
    return nc


_NC_CACHE = {}


def _get_nc(which, plan=None):
    key = (which, plan)
    if key not in _NC_CACHE:
        _NC_CACHE[key] = (build_nc_v4(plan) if which == "v4"
                          else build_nc_v1())
    return _NC_CACHE[key]


def _norm_inputs(ent_ids, current_time, hist_ids, hist_times, hist_len,
                 emb, W, b):
    return (
        np.ascontiguousarray(np.asarray(ent_ids, dtype=np.int32)),
        np.ascontiguousarray(np.asarray(current_time, np.float32)),
        np.ascontiguousarray(np.asarray(hist_ids, dtype=np.int32)),
        np.ascontiguousarray(np.asarray(hist_times, np.float32)),
        np.ascontiguousarray(np.asarray(hist_len, dtype=np.int32)),
        np.ascontiguousarray(np.asarray(emb, dtype=np.float32)),
        np.ascontiguousarray(np.asarray(W, dtype=np.float32)),
        np.ascontiguousarray(np.asarray(b, dtype=np.float32)),
    )


def make_in_maps(ent_ids, current_time, hist_ids, hist_times, hist_len,
                 emb, W, b):
    """v1 (dense) per-core input maps."""
    ent_ids, current_time, hist_ids, hist_times, hist_len, emb, W, b = \
        _norm_inputs(ent_ids, current_time, hist_ids, hist_times, hist_len,
                     emb, W, b)
    WT = np.ascontiguousarray(W.T)
    in_maps = []
    for c in range(N_CORES):
        s = slice(c * BL, (c + 1) * BL)
        in_maps.append({
            "ent_ids": ent_ids[s], "current_time": current_time[s],
            "hist_ids": hist_ids[s], "hist_times": hist_times[s],
            "hist_len": hist_len[s], "emb": emb, "WT": WT, "bvec": b,
        })
    return in_maps


def make_in_maps_v4(ent_ids, current_time, hist_ids, hist_times, hist_len,
                    emb, W, b):
    """v4 (ragged bf16) per-core input maps + output permutation + plan."""
    ent_ids, current_time, hist_ids, hist_times, hist_len, emb, W, b = \
        _norm_inputs(ent_ids, current_time, hist_ids, hist_times, hist_len,
                     emb, W, b)
    packs, order, plan = pack_v4(ent_ids, current_time, hist_ids, hist_times,
                                 hist_len)
    if packs is None:
        return None, None, None
    emb_b = _bf16(emb)
    wt_b = _bf16(W.T)
    in_maps = []
    for c in range(N_CORES):
        pk = packs[c]
        in_maps.append({
            "idxg": pk["idxg"], "idxc": pk["idxc"], "wgg": pk["wgg"],
            "segg": pk["segg"],
            "emb": emb_b, "WT": wt_b, "bvec": b,
        })
    return in_maps, order, plan


def kernel(ent_ids, current_time, hist_ids, hist_times, hist_len, emb, W, b):
    from concourse.bass_utils import run_bass_kernel_spmd

    args = (ent_ids, current_time, hist_ids, hist_times, hist_len, emb, W, b)
    in_maps, order, plan = make_in_maps_v4(*args)
    if in_maps is not None:
        nc = _get_nc("v4", plan)
        res = run_bass_kernel_spmd(nc, in_maps, list(range(N_CORES)))
        y_sorted = np.stack([
            np.asarray(res.results[c]["y"], np.float32)
            for c in range(N_CORES)])
        # core c position p holds batch row order[8p + c]
        y_full = np.empty((B, D), np.float32)
        pos = np.arange(BL)
        for c in range(N_CORES):
            y_full[order[N_CORES * pos + c]] = y_sorted[c]
        return y_full

    nc = _get_nc("v1")
    res = run_bass_kernel_spmd(nc, make_in_maps(*args),
                               list(range(N_CORES)))
    return np.concatenate([res.results[c]["y"] for c in range(N_CORES)],
                          axis=0)

